# revision 3
# baseline (speedup 1.0000x reference)
"""Trainium2 Bass kernel v5 for nn_AttrSoftLoss — PE-prefix, pipelined.

Math identical to kernel_v4 but scheduled for overlap (measured matmul cost
is ~600ns at FD=512 regardless of dtype, so the design minimizes matmul
COUNT and critical-path serialization):

 - csum_sb row 0 carries -2k directly in fp16 (even ints <= 2048 are
   fp16-exact) — no kH/kL split, two tensor_scalar ops total for k.
 - the PSUM->SBUF colsum copy runs on the otherwise-idle GpSimd so it is
   not queued behind the ScalarE Exp/Ln train.
 - all W @ h matmuls are emitted before the threshold-dependent A'' ones,
   so the PE fills psum banks while thresholds are still being computed
   (vps bufs=3: three v tiles in flight).
 - per-block FD-1024 matmuls (one per matmul kind per block).
 - h/s DMAs issue on separate queues (sync / gpsimd-dge) up front.

  v[i,r] = (W @ h_cb) + (A''_cb @ csum9)    W = triu(1) - 1025*I
         = 2*c_glob - (i+1) - 1025*h - 2k
  keep <=> v > thrp[i] = -1025 - i; stats[:,cb] = Σ_r keep * softplus(h*s)
"""

import numpy as np

B, C = 8192, 1024
N_CORES = 8
ROWS = B // N_CORES
NB = C // 128
MAGIC = 8388608.0
BIG = 1025.0

_cache: dict = {}


def _make_bacc():
    from concourse import bacc

    return bacc.Bacc(
        "TRN2", target_bir_lowering=False, debug=False, num_devices=N_CORES
    )


def _build_nc():
    from concourse import mybir, tile

    Alu = mybir.AluOpType
    Act = mybir.ActivationFunctionType
    f32 = mybir.dt.float32
    f16 = mybir.dt.float16
    R = ROWS

    nc = _make_bacc()
    # pair-interleaved transposed layout: row p of pair pb holds the two
    # class-blocks 2pb, 2pb+1 side by side -> one 4KB-line DMA per pair
    s_d = nc.dram_tensor("scores_t", [C // 2, 2 * R], f16, kind="ExternalInput")
    h_d = nc.dram_tensor("hlab_t", [C // 2, 2 * R], f16, kind="ExternalInput")
    w_d = nc.dram_tensor("wmat", [128, 128], f16, kind="ExternalInput")
    e_d = nc.dram_tensor("emat2", [128, 128], f16, kind="ExternalInput")
    a_d = nc.dram_tensor("amat", [16, 1024], f16, kind="ExternalInput")
    t_d = nc.dram_tensor("t16", [16, 1], f16, kind="ExternalInput")
    o_d = nc.dram_tensor("o16", [128, 16], f16, kind="ExternalInput")
    tp_d = nc.dram_tensor("thrp", [128, 1], f32, kind="ExternalInput")
    out_d = nc.dram_tensor("out", [1, 1], f32, kind="ExternalOutput")

    with tile.TileContext(nc) as tc:
        with (
            tc.tile_pool(name="hpool", bufs=4) as hpool,
            tc.tile_pool(name="spool", bufs=4) as spool,
            tc.tile_pool(name="work", bufs=2) as work,
            tc.tile_pool(name="sppool", bufs=4) as sppool,
            tc.tile_pool(name="stat", bufs=1) as stat,
            tc.tile_pool(name="vps", bufs=4, space="PSUM") as vps,
        ):
            wmat = stat.tile([128, 128], f16)
            emat = stat.tile([128, 128], f16)
            amat = stat.tile([16, 1024], f16)
            t16 = stat.tile([16, 1], f16)
            o16 = stat.tile([128, 16], f16)
            thrp = stat.tile([128, 1], f32)
            nc.scalar.dma_start(out=wmat[:], in_=w_d[:, :])
            nc.scalar.dma_start(out=emat[:], in_=e_d[:, :])
            nc.scalar.dma_start(out=amat[:], in_=a_d[:, :])
            nc.scalar.dma_start(out=t16[:], in_=t_d[:, :])
            nc.scalar.dma_start(out=o16[:], in_=o_d[:, :])
            nc.scalar.dma_start(out=thrp[:], in_=tp_d[:, :])

            ones16 = stat.tile([128, R], f16)
            nc.vector.memset(ones16[:], 1.0)
            onesw = stat.tile([128, 1], f32)
            nc.vector.memset(onesw[:], 1.0 / (B * C))
            stats = stat.tile([128, NB], f32)

            nc.scalar.add_instruction(
                mybir.InstLoadActFuncSet(
                    name=nc.get_next_instruction_name(),
                    act_func_set_id=6, ins=[], outs=[],
                )
            )

            csum = vps.tile([16, R], f32, tag="v", name="csum")

            # all input DMAs up front on two queues; colsums chase h arrivals
            hts, sts = [], []
            for pb in range(NB // 2):
                ht = hpool.tile([128, 2 * R], f16, tag="h", name=f"h{pb}")
                st = spool.tile([128, 2 * R], f16, tag="s", name=f"s{pb}")
                hts.append(ht)
                sts.append(st)
                heng = nc.sync if pb < 2 else nc.scalar
                heng.dma_start(
                    out=ht[:], in_=h_d[128 * pb : 128 * (pb + 1), :]
                )
                nc.gpsimd.dma_start(
                    out=st[:], in_=s_d[128 * pb : 128 * (pb + 1), :]
                )
            for cb in range(NB):
                pb, j = divmod(cb, 2)
                for rh in range(2):
                    sl = slice(512 * rh, 512 * (rh + 1))
                    nc.tensor.matmul(
                        csum[:, sl],
                        emat[:, 16 * cb : 16 * (cb + 1)],
                        hts[pb][:, R * j : R * (j + 1)][:, sl],
                        start=(cb == 0),
                        stop=False,
                    )
            for rh in range(2):
                sl = slice(512 * rh, 512 * (rh + 1))
                nc.tensor.matmul(
                    csum[:, sl], o16[:], ones16[:, sl], start=False, stop=True
                )

            # hs + softplus train (pair grain).  The thr chain (psum->sbuf
            # copy + two tensor_scalars) is emitted after pair 1 so the DVE
            # reaches it as soon as the colsums land, pulling the whole
            # A-matmul train ~5us earlier; pairs 2-3 follow it.
            spts = []

            def emit_pair(pb):
                hst = work.tile([128, 2 * R], f16, tag="hs", name=f"hs{pb}")
                nc.vector.tensor_tensor(hst[:], hts[pb][:], sts[pb][:], op=Alu.mult)
                ext = work.tile([128, 2 * R], f16, tag="ex", name=f"ex{pb}")
                nc.scalar.activation(ext[:], hst[:], Act.Exp)
                spt = sppool.tile([128, 2 * R], f16, tag="sp", name=f"sp{pb}")
                nc.scalar.activation(spt[:], ext[:], Act.Ln, bias=1.0)
                spts.append(spt)

            emit_pair(0)
            emit_pair(1)

            # csum row 0 already holds 2*n0 (colsum basis matmuls carry an
            # extra all-ones column 0, and o16 col 0 = 8.0 adds the +1024),
            # so k needs no extra matmul: two tensor_scalars off the copy.
            csum_sb = stat.tile([16, R], f16)
            nc.vector.tensor_copy(csum_sb[:], csum[:])
            t1 = stat.tile([1, R], f32)
            nc.vector.tensor_scalar(
                t1[:], csum_sb[0:1, :], 0.475, MAGIC, op0=Alu.mult, op1=Alu.add
            )
            nc.vector.tensor_scalar(
                csum_sb[0:1, :], t1[:], MAGIC, -2.0, op0=Alu.subtract, op1=Alu.mult
            )

            emit_pair(2)
            emit_pair(3)

            # masked softplus accumulate.  W matmuls are thr-independent; keep
            # exactly `bufs` of them ahead of the A/stt drain so the in-order
            # PE queue never blocks on a psum slot freed by a LATER stt.
            VB = 4  # must match vps bufs (slot 0 recycles csum after the CAST)
            vs = []

            def emit_w(cb):
                pb, j = divmod(cb, 2)
                v = vps.tile([128, R], f32, tag="v", name=f"v{cb}")
                vs.append(v)
                for rh in range(2):
                    sl = slice(512 * rh, 512 * (rh + 1))
                    nc.tensor.matmul(
                        v[:, sl], wmat[:], hts[pb][:, R * j : R * (j + 1)][:, sl],
                        start=True, stop=False,
                    )

            for cb in range(VB):
                emit_w(cb)
            for cb in range(NB):
                pb, j = divmod(cb, 2)
                for rh in range(2):
                    sl = slice(512 * rh, 512 * (rh + 1))
                    nc.tensor.matmul(
                        vs[cb][:, sl], amat[:, 128 * cb : 128 * (cb + 1)],
                        csum_sb[:, sl], start=False, stop=True,
                    )
                scr = work.tile([128, R], f16, tag="scr")
                nc.vector.scalar_tensor_tensor(
                    scr[:], vs[cb][:], thrp[:], spts[pb][:, R * j : R * (j + 1)],
                    op0=Alu.is_gt, op1=Alu.mult,
                    accum_out=stats[:, cb : cb + 1],
                )
                if cb + VB < NB:
                    emit_w(cb + VB)

            acc = stat.tile([128, 1], f32)
            nc.vector.tensor_reduce(acc[:], stats[:], mybir.AxisListType.X, Alu.add)
            part = vps.tile([1, 1], f32, tag="v", name="part")
            nc.tensor.matmul(part[:], onesw[:], acc[:], start=True, stop=True)
            res = stat.tile([1, 1], f32)
            nc.vector.tensor_copy(res[:], part[:])
            nc.sync.dma_start(out=out_d[:, :], in_=res[:])

    nc.compile()
    return nc


def _get_nc():
    if "nc" not in _cache:
        _cache["nc"] = _build_nc()
    return _cache["nc"]


def _get_perm():
    if "perm" not in _cache:
        import jax

        with jax.default_device(jax.devices("cpu")[0]):
            u = np.asarray(jax.random.uniform(jax.random.key(42), (B, C)))
        _cache["perm"] = np.argsort(u, axis=1, kind="stable")
    return _cache["perm"]


def _get_consts():
    if "consts" not in _cache:
        w = np.triu(np.ones((128, 128), np.float32)) - BIG * np.eye(
            128, dtype=np.float32
        )
        em = np.zeros((128, 128), np.float32)
        for cb in range(8):
            em[:, 16 * cb + 1 + cb] = 1.0
            em[:, 16 * cb] = 1.0
        am = np.zeros((16, 1024), np.float32)
        for cb in range(8):
            am[0, 128 * cb : 128 * (cb + 1)] = 1.0
            for b in range(cb):
                am[1 + b, 128 * cb : 128 * (cb + 1)] = 1.0
        t16 = np.zeros((16, 1), np.float32)
        t16[1:9, 0] = 1.0
        o16 = np.zeros((128, 16), np.float32)
        o16[:, 1:9] = 1.0
        o16[:, 0] = 8.0 / 128.0 * 128.0 / 128.0 * 8.0 if False else 8.0
        thrp = (-BIG - np.arange(128, dtype=np.float32)).reshape(128, 1)
        _cache["consts"] = {
            "wmat": w.astype(np.float16),
            "emat2": em.astype(np.float16),
            "amat": am.astype(np.float16),
            "t16": t16.astype(np.float16),
            "o16": o16.astype(np.float16),
            "thrp": thrp,
        }
    return _cache["consts"]


def _make_in_maps(scores: np.ndarray, attributes: np.ndarray):
    perm = _get_perm()
    s_p = np.take_along_axis(
        np.asarray(scores, dtype=np.float32), perm, axis=1
    ).astype(np.float16)
    a_p = np.take_along_axis(np.asarray(attributes, dtype=np.int32), perm, axis=1)
    h_p = (1 - 2 * a_p).astype(np.float16)
    consts = _get_consts()
    def pairpack(x_t):
        # [C, R] -> [C/2, 2R]: pair-tile pb row p = [block 2pb | block 2pb+1]
        xb = x_t.reshape(8, 128, ROWS)
        return np.ascontiguousarray(
            np.concatenate(
                [np.concatenate([xb[2 * pb], xb[2 * pb + 1]], axis=1)
                 for pb in range(4)], axis=0)
        )

    in_maps = []
    for i in range(N_CORES):
        r0, r1 = i * ROWS, (i + 1) * ROWS
        in_maps.append(
            {
                "scores_t": pairpack(s_p[r0:r1].T),
                "hlab_t": pairpack(h_p[r0:r1].T),
                **consts,
            }
        )
    return in_maps


def _run(in_maps, trace=False, **kwargs):
    from concourse import bass_utils

    return bass_utils.run_bass_kernel_spmd(
        _get_nc(), in_maps, core_ids=list(range(N_CORES)), trace=trace, **kwargs
    )


def kernel(scores: np.ndarray, attributes: np.ndarray) -> np.ndarray:
    res = _run(_make_in_maps(scores, attributes))
    parts = np.stack(
        [np.asarray(r["out"], dtype=np.float32).reshape(()) for r in res.results]
    )
    return np.float32(np.sum(parts, dtype=np.float32)).reshape(())[()]
